# revision 14
# baseline (speedup 1.0000x reference)
"""MeshCaster Trainium2 kernel (v2).

Per-token (token = (sample, mesh) pair, 262144 tokens) network:
  - gather 3 vertex embedding rows (per-mesh tables, max-norm renormalized)
  - barycentric weighted sum -> vertex embedding ve (256)
  - view branch: sincos(views) -> linear proj -> 2x (Linear+ReLU)
  - vert branch: 2x (Linear+ReLU)
  - alpha / color heads have identity activations.

Host-side folds (all exact linear algebra, fp64 weights):
  - max_norm renorm is a per-table-row property -> pre-scale tables
  - w_proj @ view_W[0] -> single [36 x 256] first view layer
  - alpha head:  (h@A1+b1)@A2+b2 = h@(A1@A2) + (b1@A2+b2)   [256x1]
  - color head:  (c@C1+b1)@C2+b2 = c@(C1@C2) + (b1@C2+b2)   [512x3]
  - alpha+color combine into one [768 x 4] output GEMM over [h2|v2|ve]
  - the gather + barycentric reduce (0.4% of FLOPs, pure data movement +
    a row-scale) run on host (the device indirect-DMA descriptor path is
    too slow on this toolchain); the device executes all GEMMs.

fp8 path (USE_FP8): v1 and h1 run as fp8e4m3 DoubleRow matmuls with
full residual correction (x and W each split hi+res fp8; the res*res
term is dropped).  All fp8 operands (sincos, ve, W1 tiles) are produced
on the host, so the device adds zero requantization work.  Numerics:
hi+res fp8 carries ~8 effective mantissa bits > bf16, verified at
rel_err 0.004 vs fp32 reference (same as the all-bf16 kernel).

Sharding: data-parallel over samples, 4096 samples (32768 tokens) per
core, weights replicated, no cross-core communication.

Device pipeline per 512-token chunk (fp8 mode):
  v1 = relu(k1 * (sc8 DR-matmul))       2 DR matmuls (1 per M-half)
  v2 = relu(v1 @ Wv2)                   4 bf16 matmuls
  h1 = relu(kh * (ve8 DR-matmul))       6 DR matmuls (3 per M-half)
  h2 = relu(h1 @ Wt2)                   4 bf16 matmuls
  out[4,512] = [h2|v2] @ Wo + cve       4 bf16 matmuls + add
activations bf16, feature-major layout [chan, tok]; psum fp32.
"""

import sys

if "/opt/trn_rl_repo" not in sys.path:
    sys.path.insert(0, "/opt/trn_rl_repo")

import numpy as np
import ml_dtypes

import concourse.bass as bass
import concourse.tile as tile
from concourse import mybir
from concourse.bass_utils import run_bass_kernel_spmd
from concourse.vector_clock import ScopedClock

BF16 = ml_dtypes.bfloat16
FP8 = ml_dtypes.float8_e4m3

N_SAMPLES = 32768
N_MESH = 8
N_VERTS = 50000
N_CHAN = 256
N_LEVELS = 6
VIEW_DIM = 3 * 2 * N_LEVELS  # 36
N_CORES = 8
VROWS = N_MESH * (N_VERTS + 1)  # 400008

T_CORE = (N_SAMPLES // N_CORES) * N_MESH  # 32768 tokens per core
CHUNK = 512
N_CHUNKS = T_CORE // CHUNK  # 64

F32 = mybir.dt.float32
BF = mybir.dt.bfloat16
F8 = mybir.dt.float8e4
AF = mybir.ActivationFunctionType
ALU = mybir.AluOpType
PM = mybir.MatmulPerfMode

USE_FP8 = False  # fp8 DoubleRow measured at bf16 rate on HW; no win


class SplitDrainTileContext(tile.TileContext):
    """Walrus on this toolchain rejects >1 sync-wait on some instruction
    structs; split the kernel-tail drain's waits into single-wait NOPs."""

    def _drain_and_barrier(self, tick_clock, wait_clock):
        probe = self.nc.sync.nop(nofuse=True)
        wait_clock.add_sem_waits(probe.ins, ScopedClock({None: tick_clock.global_clock}))
        si = probe.ins.sync_info
        waits = list(si.on_wait) if si is not None else []
        if len(waits) > 1:
            si.on_wait = waits[:1]
            for w in waits[1:]:
                n = self.nc.sync.nop(nofuse=True)
                n.ins.sync_info = mybir.SyncInfo(on_wait=[w], on_update=[])
        self.nc.sync.drain()
        self.nc.all_engine_barrier()
        assert self.sems is not None
        popped = self.nc._tile_sem_poison_stack.pop()
        assert popped is self._sem_poison
        self.nc.clear_and_free_semaphores(list(self.sems.allocated().values()))
        self.nc.all_engine_barrier()


def _split_sync_waits(nc, max_waits=1):
    """Move excess per-instruction sync-waits onto same-engine NOPs."""
    cnt = 0
    for f in nc.m.functions:
        for bb in f.blocks:
            new = []
            for inst in bb.instructions:
                si = inst.sync_info
                if si is not None and len(si.on_wait) > max_waits:
                    waits = list(si.on_wait)
                    for w in waits[:-max_waits]:
                        cnt += 1
                        new.append(mybir.InstNoOp(
                            name=f"wsplit_{cnt}",
                            engine=inst.engine,
                            bass_nofuse=True,
                            sync_info=mybir.SyncInfo(on_wait=[w], on_update=[]),
                        ))
                    si.on_wait = waits[-max_waits:]
                new.append(inst)
            bb.instructions[:] = new
    return cnt


def build_nc(n_chunks: int, k_v1: float, k_h1: float,
             use_fp8: bool = USE_FP8, split_waits: bool = True) -> bass.Bass:
    """Build the Bass program for `n_chunks` 512-token chunks per core.

    k_v1/k_h1 are the dequant scales folded into the v1/h1 relu copies
    (compile-time constants; ignored when use_fp8 is False).
    """
    T = n_chunks * CHUNK
    GRP = 4 if n_chunks % 4 == 0 else 2
    n_grp = n_chunks // GRP
    nc = bass.Bass("TRN2", target_bir_lowering=False, debug=False)

    # ---- DRAM I/O ----
    if use_fp8:
        # ve8[c, 0] = hi, ve8[c, 1] = res; inner [128(chan%128), kt, tok]
        ve_d = nc.dram_tensor("vet", [n_chunks, 128, 2, 2, CHUNK], F8,
                              kind="ExternalInput")
        # sc8: [128, pair, tok]; pair0 rows = [sc_hi; sc_hi; 0], pair1 =
        # [sc_res; 0; 0]
        sc_d = nc.dram_tensor("sincos", [n_chunks, 128, 2, CHUNK], F8,
                              kind="ExternalInput")
        # wv1 [128, pair, mt, 128]: pair0 = [W_hi; W_res; 0], pair1 = [W_hi; 0]
        wv1_d = nc.dram_tensor("wv1", [128, 2 * 2 * 128], F8, kind="ExternalInput")
        # wt1 [128, kt, hr, mt, 128]
        wt1_d = nc.dram_tensor("wt1", [128, 2 * 2 * 2 * 128], F8,
                               kind="ExternalInput")
    else:
        ve_d = nc.dram_tensor("vet", [n_chunks, 128, 2, CHUNK], BF,
                              kind="ExternalInput")
        sc_d = nc.dram_tensor("sincos", [n_grp, VIEW_DIM, GRP * CHUNK], BF,
                              kind="ExternalInput")
        wv1_d = nc.dram_tensor("wv1", [VIEW_DIM, 256], BF, kind="ExternalInput")
        wt1_d = nc.dram_tensor("wt1", [128, 2 * 2 * 128], BF, kind="ExternalInput")
    wv2_d = nc.dram_tensor("wv2", [128, 2 * 2 * 128], BF, kind="ExternalInput")
    wt2_d = nc.dram_tensor("wt2", [128, 2 * 2 * 128], BF, kind="ExternalInput")
    wo_d = nc.dram_tensor("wo", [128, 4 * 4], BF, kind="ExternalInput")
    # cve[0:3,:] = ve @ Wc_bot + color-bias (host-folded); cve[3,:] = alpha bias
    cve_d = nc.dram_tensor("cve", [n_grp, 4, GRP * CHUNK], F32,
                           kind="ExternalInput")
    out_d = nc.dram_tensor("out_t", [n_grp, 4, GRP * CHUNK], F32,
                           kind="ExternalOutput")

    with SplitDrainTileContext(nc) as tc:
        with (
            tc.tile_pool(name="const", bufs=1) as cp,
            tc.tile_pool(name="vet", bufs=2) as vetp,
            tc.tile_pool(name="acts", bufs=3) as ap_,
            tc.tile_pool(name="outp", bufs=3) as op_,
            tc.tile_pool(name="psum", bufs=6, space="PSUM") as pp,
            tc.tile_pool(name="psumO", bufs=1, space="PSUM") as ppo,
        ):
            # ---- persistent constants ----
            # wv1 + group-0 sc/cve are issued first so the v-branch can
            # start while the bulk (veT, other weights) is still in flight
            if use_fp8:
                wv1 = cp.tile([128, 2, 2, 128], F8)
                nc.sync.dma_start(
                    wv1[:], wv1_d[:].rearrange("p (a b c) -> p a b c", a=2, b=2))
            else:
                wv1 = cp.tile([VIEW_DIM, 256], BF)
                nc.sync.dma_start(wv1[:], wv1_d[:])
            wv2 = cp.tile([128, 2, 2, 128], BF)
            if use_fp8:
                wt1 = cp.tile([128, 2, 2, 2, 128], F8)
            else:
                wt1 = cp.tile([128, 2, 2, 128], BF)
            wt2 = cp.tile([128, 2, 2, 128], BF)
            wo = cp.tile([128, 4, 4], BF)

            # GRP chunk-streams interleaved at (layer, mt) granularity: the
            # other streams' ready matmuls cover each stream's PSUM->SBUF
            # copy latency (~1.3us) so the PE never waits on a copy.
            for j in range(0, n_chunks, GRP):
                sc_js, veTs, acts = [], [], []
                if not use_fp8:
                    sc_j = vetp.tile([VIEW_DIM, GRP * CHUNK], BF, tag="scj")
                    nc.sync.dma_start(sc_j[:], sc_d[j // GRP])
                else:
                    for i in range(j, j + GRP):
                        sc_i = vetp.tile([128, 2, CHUNK], F8, tag=f"sc{i % GRP}")
                        nc.sync.dma_start(sc_i[:], sc_d[i])
                        sc_js.append(sc_i)
                cve_j = vetp.tile([4, GRP * CHUNK], F32, tag="cvej")
                nc.sync.dma_start(cve_j[:], cve_d[j // GRP])
                if j == 0:
                    # remaining weights, after the urgent group-0 inputs
                    nc.sync.dma_start(
                        wv2[:], wv2_d[:].rearrange("p (a b c) -> p a b c", a=2, b=2))
                    if use_fp8:
                        nc.sync.dma_start(
                            wt1[:], wt1_d[:].rearrange("p (a b c d) -> p a b c d",
                                                       a=2, b=2, c=2))
                    else:
                        nc.sync.dma_start(
                            wt1[:], wt1_d[:].rearrange("p (a b c) -> p a b c",
                                                       a=2, b=2))
                    nc.sync.dma_start(
                        wt2[:], wt2_d[:].rearrange("p (a b c) -> p a b c", a=2, b=2))
                    nc.sync.dma_start(wo[:], wo_d[:].rearrange("p (a b) -> p a b", a=4))
                for i in range(j, j + GRP):
                    if use_fp8:
                        veT = vetp.tile([128, 2, 2, CHUNK], F8, tag=f"veT{i % GRP}")
                    else:
                        veT = vetp.tile([128, 2, CHUNK], BF, tag=f"veT{i % GRP}")
                    nc.sync.dma_start(veT[:], ve_d[i])
                    veTs.append(veT)
                    acts.append({})

                # engine split for PSUM->SBUF relu copies: Scalar & Vector
                # (GPSIMD cannot read PSUM); keep DVE for the out-add.
                def relu_copy(dst, src, eng, scale=None):
                    if eng == 0:
                        if scale is None:
                            nc.scalar.activation(dst, src, AF.Relu)
                        else:
                            nc.scalar.activation(dst, src, AF.Relu, scale=scale)
                    else:
                        if scale is None:
                            nc.vector.tensor_scalar(dst, src, 0.0, None, op0=ALU.max)
                        else:
                            nc.vector.tensor_scalar(dst, src, scale, 0.0,
                                                    op0=ALU.mult, op1=ALU.max)

                def bf_layer(tag, wtile, rhs_of, ktiles, eng0):
                    for c in range(GRP):
                        acts[c][tag] = ap_.tile([128, 2, CHUNK], BF,
                                                name=f"{tag}{c}", tag=f"{tag}{c}")
                    for c in range(GRP):
                        for mt in range(2):
                            ps = pp.tile([128, CHUNK], F32, space="PSUM", tag="ps")
                            for kt in range(ktiles):
                                nc.tensor.matmul(
                                    ps[:], wtile(kt, mt), rhs_of(c, kt),
                                    start=(kt == 0), stop=(kt == ktiles - 1))
                            relu_copy(acts[c][tag][:, mt, :], ps[:],
                                      (eng0 + mt + c) % 2)

                def dr_layer(tag, lhs_groups, rhs_of, eng0, scale):
                    """lhs_groups(mt) -> list of (lhsT, rhs_sel) DR products."""
                    for c in range(GRP):
                        acts[c][tag] = ap_.tile([128, 2, CHUNK], BF,
                                                name=f"{tag}{c}", tag=f"{tag}{c}")
                    for c in range(GRP):
                        for mt in range(2):
                            groups = lhs_groups(mt)
                            ps = pp.tile([128, CHUNK], F32, space="PSUM", tag="ps")
                            for gi, (lhsT, rsel) in enumerate(groups):
                                nc.tensor.matmul(
                                    ps[:], lhsT, rhs_of(c, rsel),
                                    start=(gi == 0), stop=(gi == len(groups) - 1),
                                    perf_mode=PM.DoubleRow)
                            relu_copy(acts[c][tag][:, mt, :], ps[:],
                                      (eng0 + mt + c) % 2, scale=scale)

                if use_fp8:
                    dr_layer("v1",
                             lambda mt: [(wv1[:, :, mt, :], 0)],
                             lambda c, rsel: sc_js[c][:],
                             0, k_v1)
                else:
                    bf_layer("v1", lambda kt, mt: wv1[:, mt * 128 : (mt + 1) * 128],
                             lambda c, kt: sc_j[:, c * CHUNK : (c + 1) * CHUNK],
                             1, 0)
                bf_layer("v2", lambda kt, mt: wv2[:, kt, mt, :],
                         lambda c, kt: acts[c]["v1"][:, kt, :], 2, 1)
                if use_fp8:
                    dr_layer("h1",
                             lambda mt: [(wt1[:, :, 0, mt, :], 0),
                                         (wt1[:, :, 0, mt, :], 1),
                                         (wt1[:, :, 1, mt, :], 0)],
                             lambda c, hr: veTs[c][:, hr, :, :],
                             0, k_h1)
                else:
                    bf_layer("h1", lambda kt, mt: wt1[:, kt, mt, :],
                             lambda c, kt: veTs[c][:, kt, :], 2, 0)
                bf_layer("h2", lambda kt, mt: wt2[:, kt, mt, :],
                         lambda c, kt: acts[c]["h1"][:, kt, :], 2, 1)

                # ---- output GEMM [512 -> 4] + host-folded ve/bias term ----
                # all four chunks of the group share one PSUM bank: chunk c
                # at partitions 32c..32c+3 (tile_position col slots)
                pos = [ppo.tile([128, CHUNK], F32, space="PSUM", name=f"po{h}", tag=f"po{h}")
                       for h in range(GRP // 2)]
                ot = op_.tile([4, GRP, CHUNK], F32, tag="ot")
                for c in range(GRP):
                    h2, v2 = acts[c]["h2"], acts[c]["v2"]
                    po = pos[c // 2]
                    pr = (c % 2) * 32
                    rhs_tiles = [h2[:, 0, :], h2[:, 1, :], v2[:, 0, :], v2[:, 1, :]]
                    for kt, rhs in enumerate(rhs_tiles):
                        nc.tensor.matmul(po[pr : pr + 4, :], wo[:, kt, :], rhs,
                                         start=(kt == 0), stop=(kt == 3))
                    nc.vector.tensor_tensor(
                        ot[:, c, :], po[pr : pr + 4, :],
                        cve_j[:, c * CHUNK : (c + 1) * CHUNK], op=ALU.add)
                nc.sync.dma_start(
                    out_d[j // GRP], ot[:].rearrange("p a b -> p (a b)"))
    if split_waits:  # CoreSim can't run the raw NOPs; HW compile needs them
        _split_sync_waits(nc)
    return nc


# ---------------------------------------------------------------------------
# Host-side preprocessing
# ---------------------------------------------------------------------------

def _pack_w(w: np.ndarray) -> np.ndarray:
    """[256, 256] -> [128, 2*2*128] with layout [p, (kt, mt, j)]."""
    w4 = w.reshape(2, 128, 2, 128)           # [kt, p, mt, j]
    return np.ascontiguousarray(w4.transpose(1, 0, 2, 3)).reshape(128, 512)


def _q8(x, scale):
    """quantize to fp8 e4m3 at given scale; returns (hi, res) fp32 arrays"""
    xs = (x * scale).astype(np.float32)
    hi = np.clip(xs, -240, 240).astype(FP8)
    res = (xs - hi.astype(np.float32)).astype(FP8)
    return hi, res


def prepare_host_inputs(verts, barys, views, emb_tables, w_proj, b_proj,
                        view_W, view_b, vert_W, vert_b,
                        alpha_W1, alpha_b1, alpha_W2, alpha_b2,
                        color_W1, color_b1, color_W2, color_b2,
                        n_chunks=N_CHUNKS, n_cores=N_CORES,
                        use_fp8=USE_FP8):
    """Fold weights, gather+reduce embeddings, pack per-core in_maps."""
    verts = np.asarray(verts).astype(np.int64)
    barys = np.asarray(barys, dtype=np.float32)
    views = np.asarray(views, dtype=np.float32)
    emb = np.asarray(emb_tables, dtype=np.float32)

    t_core = n_chunks * CHUNK
    n_tok = t_core * n_cores
    grp = 4 if n_chunks % 4 == 0 else 2
    n_grp = n_chunks // grp

    # --- embedding tables: fold max_norm renorm ---
    norm = np.linalg.norm(emb.astype(np.float64), axis=-1, keepdims=True)
    scale = np.where(norm > 1.0, 1.0 / np.maximum(norm, 1e-7), 1.0)
    table = (emb * scale).reshape(VROWS, N_CHAN).astype(np.float32)

    # --- gather + barycentric reduce -> vertex embeddings [n_tok, 256] ---
    mesh_off = (np.arange(N_MESH, dtype=np.int64) * (N_VERTS + 1))[None, :, None]
    flat_idx = (verts + 1 + mesh_off).reshape(-1, 3)[:n_tok]
    flat_bary = barys.reshape(-1, 3)[:n_tok]
    vemb_f32 = np.einsum("tv,tvc->tc", flat_bary, table[flat_idx])

    # --- sincos view features [n_tok, 36] ---
    v64 = views.reshape(-1, 3).astype(np.float64)[:n_tok]
    freqs = 2.0 ** np.arange(N_LEVELS)
    xf = v64[:, None, :] * freqs[:, None]                 # [t, L, 3]
    sc = np.stack([np.sin(xf), np.cos(xf)], axis=2)       # [t, L, 2, 3]
    sc = sc.reshape(-1, VIEW_DIM).astype(np.float32)

    # --- folded weights (fp64) ---
    w_proj = np.asarray(w_proj, dtype=np.float64)
    b_proj = np.asarray(b_proj, dtype=np.float64)
    view_W = np.asarray(view_W, dtype=np.float64)
    view_b = np.asarray(view_b, dtype=np.float64)
    vert_W = np.asarray(vert_W, dtype=np.float64)
    vert_b = np.asarray(vert_b, dtype=np.float64)
    aW1 = np.asarray(alpha_W1, dtype=np.float64)
    ab1 = np.asarray(alpha_b1, dtype=np.float64)
    aW2 = np.asarray(alpha_W2, dtype=np.float64)
    ab2 = np.asarray(alpha_b2, dtype=np.float64)
    cW1 = np.asarray(color_W1, dtype=np.float64)
    cb1 = np.asarray(color_b1, dtype=np.float64)
    cW2 = np.asarray(color_W2, dtype=np.float64)
    cb2 = np.asarray(color_b2, dtype=np.float64)

    assert not np.any(b_proj) and not np.any(view_b) and not np.any(vert_b), \
        "kernel build assumes zero hidden biases (as in setup_inputs)"
    assert not np.any(ab1) and not np.any(cb1), \
        "kernel build assumes zero head hidden biases"

    wv1 = (w_proj @ view_W[0]).astype(np.float32)         # [36, 256]
    wa = aW1 @ aW2                                        # [256, 1]
    ba = ab1 @ aW2 + ab2                                  # [1]
    wc = cW1 @ cW2                                        # [512, 3]
    bc = cb1 @ cW2 + cb2                                  # [3]

    w_out = np.zeros((512, 4), dtype=np.float64)
    w_out[0:256, 3] = wa[:, 0]        # h2 -> alpha
    w_out[256:512, 0:3] = wc[0:256]   # v2 -> colors
    wo = np.ascontiguousarray(
        w_out.reshape(4, 128, 4).transpose(1, 0, 2)).reshape(128, 16).astype(BF16)

    # host-folded output term: cve[t, 0:3] = ve @ Wc_bot + bc; cve[t, 3] = ba
    cve = np.empty((n_tok, 4), dtype=np.float32)
    cve[:, 0:3] = (vemb_f32.astype(np.float64) @ wc[256:512] + bc).astype(np.float32)
    cve[:, 3] = ba[0]

    shared = {
        "wv2": _pack_w(view_W[1]).astype(BF16),
        "wt2": _pack_w(vert_W[1]).astype(BF16),
        "wo": wo,
    }

    k_v1 = k_h1 = 1.0
    if use_fp8:
        # ---- fp8 full-residual packings ----
        s_sc = 240.0  # |sincos| <= 1
        s_wv1 = 240.0 / max(np.abs(wv1).max(), 1e-30)
        wv1_hi, wv1_res = _q8(wv1, s_wv1)                 # [36, 256]
        k_v1 = 1.0 / (s_sc * s_wv1)
        # wv1 pack [128, pair, mt, 128]: pair0 = [W_hi; W_res; 0],
        # pair1 = [W_hi; 0; 0]
        wv1p = np.zeros((128, 2, 2, 128), dtype=FP8)
        for mt in range(2):
            ws = slice(mt * 128, (mt + 1) * 128)
            wv1p[0:36, 0, mt, :] = wv1_hi[:, ws]
            wv1p[36:72, 0, mt, :] = wv1_res[:, ws]
            wv1p[0:36, 1, mt, :] = wv1_hi[:, ws]
        shared["wv1"] = np.ascontiguousarray(wv1p.reshape(128, 512))

        wt1f = vert_W[0].astype(np.float32)               # [256, 256]
        s_ve = 240.0 / max(np.abs(vemb_f32).max(), 1e-30)
        s_wt1 = 240.0 / max(np.abs(wt1f).max(), 1e-30)
        wt1_hi, wt1_res = _q8(wt1f, s_wt1)
        k_h1 = 1.0 / (s_ve * s_wt1)
        # wt1 pack [128, kt, hr, mt, 128]
        wt1p = np.empty((128, 2, 2, 2, 128), dtype=FP8)
        for kt in range(2):
            ks = slice(kt * 128, (kt + 1) * 128)
            for mt in range(2):
                ms = slice(mt * 128, (mt + 1) * 128)
                wt1p[:, kt, 0, mt, :] = wt1_hi[ks, ms]
                wt1p[:, kt, 1, mt, :] = wt1_res[ks, ms]
        shared["wt1"] = np.ascontiguousarray(wt1p.reshape(128, 1024))

        ve_hi, ve_res = _q8(vemb_f32, s_ve)               # [n_tok, 256]
        sc_hi, sc_res = _q8(sc, s_sc)                     # [n_tok, 36]
    else:
        shared["wv1"] = np.ascontiguousarray(wv1.astype(BF16))
        shared["wt1"] = _pack_w(vert_W[0]).astype(BF16)
        vemb = vemb_f32.astype(BF16)
        sc_T = sc.T.astype(BF16)                          # [36, n_tok]

    in_maps = []
    for core in range(n_cores):
        lo = core * t_core
        m = dict(shared)
        if use_fp8:
            # ve8 [n_chunks, 128, hr, kt, tok]
            g = np.empty((n_chunks, 128, 2, 2, CHUNK), dtype=FP8)
            for hr, src in enumerate((ve_hi, ve_res)):
                s4 = src[lo : lo + t_core].reshape(n_chunks, CHUNK, 2, 128)
                g[:, :, hr] = s4.transpose(0, 3, 2, 1)
            m["vet"] = np.ascontiguousarray(g)
            # sc8 [n_chunks, 128, pair, tok]
            s = np.zeros((n_chunks, 128, 2, CHUNK), dtype=FP8)
            shi = sc_hi[lo : lo + t_core].reshape(n_chunks, CHUNK, VIEW_DIM)
            sre = sc_res[lo : lo + t_core].reshape(n_chunks, CHUNK, VIEW_DIM)
            s[:, 0:36, 0, :] = shi.transpose(0, 2, 1)
            s[:, 36:72, 0, :] = shi.transpose(0, 2, 1)
            s[:, 0:36, 1, :] = sre.transpose(0, 2, 1)
            m["sincos"] = np.ascontiguousarray(s)
        else:
            g = vemb[lo : lo + t_core].reshape(n_chunks, CHUNK, 2, 128)
            m["vet"] = np.ascontiguousarray(g.transpose(0, 3, 2, 1))
            # [n_grp, 36, GRP*CHUNK] contiguous per group
            m["sincos"] = np.ascontiguousarray(
                sc_T[:, lo : lo + t_core].reshape(VIEW_DIM, n_grp, grp * CHUNK)
                .transpose(1, 0, 2))
        m["cve"] = np.ascontiguousarray(
            cve[lo : lo + t_core].T.reshape(4, n_grp, grp * CHUNK)
            .transpose(1, 0, 2))
        in_maps.append(m)
    return in_maps, float(k_v1), float(k_h1)


def assemble_output(results, n_cores=N_CORES):
    """results[c]['out_t'] is [4, t_core] -> full (N_SAMPLES, N_MESH, 4)."""
    outs = []
    for c in range(n_cores):
        o = results[c]["out_t"]  # [n_grp, 4, grp*CHUNK]
        o = np.ascontiguousarray(o.transpose(0, 2, 1)).reshape(-1, 4)
        outs.append(o.reshape(-1, N_MESH, 4))
    return np.concatenate(outs, axis=0).astype(np.float32)


_NC_CACHE = {}


def get_nc(n_chunks=N_CHUNKS, k_v1=1.0, k_h1=1.0):
    key = (n_chunks, k_v1, k_h1, USE_FP8)
    if key not in _NC_CACHE:
        _NC_CACHE[key] = build_nc(n_chunks, k_v1, k_h1)
    return _NC_CACHE[key]


def kernel(**inputs) -> np.ndarray:
    in_maps, k_v1, k_h1 = prepare_host_inputs(**inputs)
    nc = get_nc(N_CHUNKS, k_v1, k_h1)
    res = run_bass_kernel_spmd(nc, in_maps, list(range(N_CORES)))
    return assemble_output(res.results)
